# revision 1
# baseline (speedup 1.0000x reference)
"""Trainium2 Bass kernel for nn_CrossAttention_45466523796037.

Per-token cross attention: q/k/v projections (1024->1024), per-token 16x16
attention over heads (contraction over head_dim=64, softmax over heads),
attn @ v, output projection with bias.  xpos/ypos are unused (rope=None).

Sharding: data-parallel over batch B=8 -> one batch per NeuronCore.

Layout strategy (per core, N=2048 tokens, C=1024):
 - host passes x.T tiles so projections run with stationary = x.T tile,
   moving = W.T -> q/k/v arrive in [token-partition, channel-free] layout.
 - middle stage (logits/softmax/attn.v) runs on DVE/ACT with tokens on
   partitions (128 tokens per tile, 16 tiles).
 - the reference's faithful-to-torch quirk `transpose(0,2,1,3).reshape(B,N,C)`
   maps x[n, h, d] -> X'[n', c'] with n' = h*128 + n//16, c' = (n%16)*64 + d
   (a cross-token shuffle).  We PE-transpose X per token tile into
   XT[(h,d), (i, t)] and then run the output projection per OUTPUT tile h
   as 16 K=64 matmuls whose stationary operands are strided views of XT
   (no extra data movement).  Wp.T is stored duplicated on both partition
   parities so the moving operand's partition base can track the
   stationary's (h%2) base.  Bias is folded in via a K=1 ones matmul into
   the same PSUM accumulation group.
"""

import sys
import os

sys.path.insert(0, "/opt/trn_rl_repo")

import numpy as np
import ml_dtypes

import concourse.bass as bass
import concourse.bacc as bacc
import concourse.mybir as mybir
import concourse.tile as tile
from concourse.bass_utils import run_bass_kernel_spmd

# problem constants (hardcoded per contract)
B, N, C = 8, 2048, 1024
H, D = 16, 64
SCALE = D ** -0.5
NT = N // 128          # 16 token tiles per core
CT = C // 128          # 8 contraction tiles
F32 = mybir.dt.float32
BF16 = mybir.dt.bfloat16
BF = ml_dtypes.bfloat16

ts = bass.ts


def build_kernel(nt: int = NT):
    """Build the per-core kernel for `nt` token tiles (nt=NT for real runs,
    smaller for simulation)."""
    n = nt * 128
    nc = bacc.Bacc("TRN2", target_bir_lowering=False, debug=False, num_devices=8)

    # DRAM I/O (per core)
    xq = nc.dram_tensor("xq", [nt, 128, CT, 128], BF16, kind="ExternalInput")
    xk = nc.dram_tensor("xk", [nt, 128, CT, 128], BF16, kind="ExternalInput")
    xv = nc.dram_tensor("xv", [nt, 128, CT, 128], BF16, kind="ExternalInput")
    wq = nc.dram_tensor("wq", [CT, 128, C], BF16, kind="ExternalInput")
    wk = nc.dram_tensor("wk", [CT, 128, C], BF16, kind="ExternalInput")
    wv = nc.dram_tensor("wv", [CT, 128, C], BF16, kind="ExternalInput")
    # wp duplicated on both partition parities: wp[u] = [Wp.T rows u*64..; same]
    wp = nc.dram_tensor("wp", [H, 128, C], BF16, kind="ExternalInput")
    bp = nc.dram_tensor("bp", [1, C], F32, kind="ExternalInput")
    ones1 = nc.dram_tensor("ones1", [1, 128], F32, kind="ExternalInput")
    ident = nc.dram_tensor("ident", [128, 128], F32, kind="ExternalInput")
    out = nc.dram_tensor("out", [n, C], F32, kind="ExternalOutput")

    with tile.TileContext(nc) as tc:
        with (
            tc.tile_pool(name="weights", bufs=1) as wpool,
            tc.tile_pool(name="xin", bufs=2) as xpool,
            tc.tile_pool(name="qkv", bufs=2) as qkvpool,
            tc.tile_pool(name="mid", bufs=3) as midpool,
            tc.tile_pool(name="prod", bufs=3) as prodpool,
            tc.tile_pool(name="osb", bufs=2) as opool,
            tc.tile_pool(name="ps_proj", bufs=4, space="PSUM") as ps_proj,
            tc.tile_pool(name="ps_xt", bufs=2, space="PSUM") as ps_xt,
            tc.tile_pool(name="ps_o", bufs=2, space="PSUM") as ps_o,
        ):
            # ---- preload weights / constants ----
            wq_sb = wpool.tile([128, CT, C], BF16, tag="wq")
            wk_sb = wpool.tile([128, CT, C], BF16, tag="wk")
            wv_sb = wpool.tile([128, CT, C], BF16, tag="wv")
            wp_sb = wpool.tile([128, H, C], BF16, tag="wp")
            bp_sb = wpool.tile([1, C], F32, tag="bp")
            ones_sb = wpool.tile([1, 128], F32, tag="ones")
            id_sb = wpool.tile([128, 128], F32, tag="ident")
            for ci in range(CT):
                nc.sync.dma_start(wq_sb[:, ci, :], wq[ci])
                nc.sync.dma_start(wk_sb[:, ci, :], wk[ci])
                nc.sync.dma_start(wv_sb[:, ci, :], wv[ci])
            for u in range(H):
                nc.sync.dma_start(wp_sb[:, u, :], wp[u])
            nc.sync.dma_start(bp_sb[:], bp[:])
            nc.sync.dma_start(ones_sb[:], ones1[:])
            nc.sync.dma_start(id_sb[:], ident[:])

            # persistent attention-output transpose: XT_j[(h2,d), i, t]
            # holds x[128*i + t, h*64 + d] for h = 2*j + h2//? (h-pair j)
            xt_all = [wpool.tile([128, nt, 128], BF16, tag=f"xt{j}",
                                 name=f"xt{j}")
                      for j in range(CT)]

            for i in range(nt):
                # ---- load x.T tiles for this token tile ----
                xq_sb = xpool.tile([128, CT, 128], BF16, tag="xq")
                xk_sb = xpool.tile([128, CT, 128], BF16, tag="xk")
                xv_sb = xpool.tile([128, CT, 128], BF16, tag="xv")
                nc.sync.dma_start(xq_sb[:], xq[i])
                nc.sync.dma_start(xk_sb[:], xk[i])
                nc.sync.dma_start(xv_sb[:], xv[i])

                # ---- projections: q/k/v in [token-part, c-free] ----
                q_sb = qkvpool.tile([128, C], F32, tag="q")
                k_sb = qkvpool.tile([128, C], F32, tag="k")
                v_sb = qkvpool.tile([128, C], F32, tag="v")
                for (x_sb, w_sb, dst) in (
                    (xq_sb, wq_sb, q_sb),
                    (xk_sb, wk_sb, k_sb),
                    (xv_sb, wv_sb, v_sb),
                ):
                    for co in range(2):
                        psum = ps_proj.tile([128, 512], F32, tag="proj")
                        for ci in range(CT):
                            nc.tensor.matmul(
                                psum[:],
                                x_sb[:, ci, :],
                                w_sb[:, ci, ts(co, 512)],
                                start=(ci == 0),
                                stop=(ci == CT - 1),
                            )
                        nc.scalar.copy(dst[:, ts(co, 512)], psum[:])

                # ---- logits: L[n, h, g] = sum_d q[n,h,d] k[n,g,d] ----
                q3 = q_sb[:].rearrange("p (h d) -> p h d", d=D)
                L = midpool.tile([128, H, H], F32, tag="L")  # (h, g)
                for g in range(H):
                    prod = prodpool.tile([128, H, D], F32, tag="prod")
                    kg = k_sb[:, ts(g, D)].unsqueeze(1).broadcast_to([128, H, D])
                    nc.vector.scalar_tensor_tensor(
                        prod[:], q3, 1.0, kg,
                        op0=mybir.AluOpType.mult, op1=mybir.AluOpType.mult,
                    )
                    nc.vector.reduce_sum(
                        L[:, :, g], prod[:], axis=mybir.AxisListType.X
                    )

                # ---- softmax over g (fold SCALE into exp) ----
                E = midpool.tile([128, H, H], F32, tag="E")
                nc.scalar.activation(
                    E[:].rearrange("p h g -> p (h g)"),
                    L[:].rearrange("p h g -> p (h g)"),
                    mybir.ActivationFunctionType.Exp,
                    scale=SCALE,
                )
                S = midpool.tile([128, H], F32, tag="S")
                nc.vector.reduce_sum(S[:], E[:], axis=mybir.AxisListType.X)
                R = midpool.tile([128, H], F32, tag="R")
                nc.vector.reciprocal(R[:], S[:])
                A = midpool.tile([128, H, H], F32, tag="A")
                rb = R[:].unsqueeze(2).broadcast_to([128, H, H])
                nc.vector.scalar_tensor_tensor(
                    A[:], E[:], 1.0, rb,
                    op0=mybir.AluOpType.mult, op1=mybir.AluOpType.mult,
                )

                # ---- attn @ v: X[n, h, d] = sum_g A[n,h,g] v[n,g,d] ----
                X = midpool.tile([128, C], F32, tag="X")
                X3 = X[:].rearrange("p (h d) -> p h d", d=D)
                for g in range(H):
                    vg = v_sb[:, ts(g, D)].unsqueeze(1).broadcast_to([128, H, D])
                    ag = A[:, :, g].unsqueeze(2).broadcast_to([128, H, D])
                    if g == 0:
                        nc.vector.scalar_tensor_tensor(
                            X3, vg, 1.0, ag,
                            op0=mybir.AluOpType.mult, op1=mybir.AluOpType.mult,
                        )
                    else:
                        pg = prodpool.tile([128, H, D], F32, tag="prod")
                        nc.vector.scalar_tensor_tensor(
                            pg[:], vg, 1.0, ag,
                            op0=mybir.AluOpType.mult, op1=mybir.AluOpType.mult,
                        )
                        nc.vector.scalar_tensor_tensor(
                            X3, pg[:], 0.0, X3,
                            op0=mybir.AluOpType.add, op1=mybir.AluOpType.add,
                        )

                # ---- transpose X into persistent XT tiles (bf16) ----
                for jj in range(2):
                    ps_t = ps_xt.tile([128, 512], F32, tag="xt")
                    for j in range(4):
                        nc.tensor.transpose(
                            ps_t[:, ts(j, 128)],
                            X[:, ts(jj * 4 + j, 128)],
                            id_sb[:],
                        )
                    for j in range(4):
                        nc.scalar.copy(
                            xt_all[jj * 4 + j][:, i, :], ps_t[:, ts(j, 128)]
                        )

            # ---- phase 2: output projection per OUTPUT tile (head h) ----
            # O[h*nt*8 + 8i + s, c_o] = bias + sum_u xhat[.,u-block] @ WpT
            # stationary_u = XT_{h//2}[(h%2)*64+d, i, u::16]  (shape [64, nt, 8])
            M = nt * 8
            for h in range(H):
                j, par = h // 2, (h % 2) * 64
                o_sb = opool.tile([M, C], F32, tag="o")
                for co in range(2):
                    psum = ps_o.tile([M, 512], F32, tag="o")
                    nc.tensor.matmul(
                        psum[:],
                        ones_sb[:, :M],
                        bp_sb[:, ts(co, 512)],
                        start=True,
                        stop=False,
                        skip_group_check=True,
                    )
                    for u in range(H):
                        lhsT = xt_all[j][par:par + 64, :, u::16]
                        rhs = wp_sb[par:par + 64, u, ts(co, 512)]
                        nc.tensor.matmul(
                            psum[:],
                            lhsT,
                            rhs,
                            start=False,
                            stop=(u == H - 1),
                            skip_group_check=True,
                        )
                    nc.scalar.copy(o_sb[:, ts(co, 512)], psum[:])
                nc.sync.dma_start(out[h * M:(h + 1) * M, :], o_sb[:])

    nc.compile()
    return nc


def prep_core_inputs(q_b: np.ndarray, k_b: np.ndarray, v_b: np.ndarray,
                     shared: dict) -> dict:
    """Host-side layout prep for one core (batch)."""
    def tiles(x):
        # [N, C] -> [nt, 128, CT, 128] with [i, c, ci, n] = x[i*128+n, ci*128+c]
        return np.ascontiguousarray(
            x.reshape(NT, 128, CT, 128).transpose(0, 3, 2, 1)
        ).astype(BF)

    m = {"xq": tiles(q_b), "xk": tiles(k_b), "xv": tiles(v_b)}
    m.update(shared)
    return m


def wp_dup(Wp: np.ndarray) -> np.ndarray:
    """[H, 128, C]: slot u = Wp.T rows u*64..(u+1)*64 duplicated on both
    partition parities so the moving operand can match the stationary's
    partition base."""
    wpt = np.float32(Wp).T.reshape(H, 64, C)
    return np.ascontiguousarray(
        np.concatenate([wpt, wpt], axis=1)
    ).astype(BF)


_NC_CACHE = {}
_TRACE = False  # test harness sets this for neuron-profile timing


def kernel(**inputs) -> np.ndarray:
    query = np.asarray(inputs["query"], np.float32)
    key_ = np.asarray(inputs["key"], np.float32)
    value = np.asarray(inputs["value"], np.float32)
    Wq = np.asarray(inputs["Wq"], np.float32)
    Wk = np.asarray(inputs["Wk"], np.float32)
    Wv = np.asarray(inputs["Wv"], np.float32)
    Wp = np.asarray(inputs["Wp"], np.float32)
    bp = np.asarray(inputs["bp"], np.float32)

    if "nc" not in _NC_CACHE:
        _NC_CACHE["nc"] = build_kernel(NT)
    nc = _NC_CACHE["nc"]

    def wtiles(W):
        return np.ascontiguousarray(W.T.reshape(CT, 128, C)).astype(BF)

    shared = {
        "wq": wtiles(Wq), "wk": wtiles(Wk), "wv": wtiles(Wv),
        "wp": wp_dup(Wp),
        "bp": bp.reshape(1, C).astype(np.float32),
        "ones1": np.ones((1, 128), np.float32),
        "ident": np.eye(128, dtype=np.float32),
    }
    in_maps = [
        prep_core_inputs(query[b], key_[b], value[b], shared) for b in range(B)
    ]
    res = run_bass_kernel_spmd(nc, in_maps, list(range(B)), trace=_TRACE)
    _NC_CACHE["last_res"] = res
    out = np.stack([res.results[b]["out"] for b in range(B)], axis=0)
    return out.astype(np.float32)

